# revision 33
# baseline (speedup 1.0000x reference)
"""Multi-head attention (B=2, T=2048, F=1024, H=16) on 8 trn2 NeuronCores.

Sharding: hybrid batch x head-group. Core c handles batch c//4 and head
group c%4 (4 heads = 2 pairs of 2). Each core reads only its batch's
activations, projects with its 256-column weight slices, runs attention
for its 4 heads, and row-sliced output projection producing a partial
(T, F) output; the host sums 4 partials per batch and adds bo.

Layout: everything is computed transposed (Q^T, K^T, V^T, S^T = K Q^T,
ctx^T); the only on-chip transposes are 16 cheap 128x128 PE transposes per
pair to build token-major V for the PV matmul. A ones-column appended to V
makes the softmax denominator fall out of the PV matmul for free;
normalization is deferred to after PV (it scales matmul columns linearly).

Machine balance: ScalarE's exp over the 16.8M-element score matrix
(~147us busy, 1 elem/lane/cycle @1.2GHz, no other engine can exp) is the
hard floor; PE needs ~140us. The schedule keeps the exp stream
back-to-back: the first exp fires once wk+wq+the first 512-token chunks
of xk/xq land (~12us); every other input DMA, projection, v1 transpose,
normalization and output-projection op is dripped into the ~500ns/iter
PE slack under exp, paced so no matmul ever enters the in-order PE queue
before its data can be there (a stalled matmul blocks the S stream and
starves the exp). After the first exp the ScalarE queue carries nothing
but ACTIVATEs. PE HAM warm-up dummies run during the initial DMA wait.

PSUM budget (8 banks): score pairs 2x2 (double-buffered [128,1024]),
ctx0/ctx1 accumulators 2, one long-lived bank ("pp") for dripped
projection accumulation -- only one projection chunk is ever in flight --
and one short-lived bank ("ps") for transposes/norm/output-projection.

PE utilization: the S matmul contracts over only 64 head dims, so the two
heads of a pair are issued at tile_position (0,0)/(64,0) and run
concurrently; both heads' scores land in one [128, 1024] PSUM tile so a
single exp covers the pair.

MODE selects the matmul operand dtype: fp16 (default; ~5e-4 operand
rounding, full PE rate, half DMA), bf16, or f32r.
"""

import os
from collections import deque

import numpy as np

import concourse.mybir as mybir
import concourse.tile as tile
from concourse import bacc
from concourse.bass_utils import run_bass_kernel_spmd

B, T, F = 2, 2048, 1024
H, DK = 16, 64
NCORES = 8
NGROUPS = 4                # head groups (one per core within a batch)
NPAIRS = 2                 # head pairs per core
HD = 2 * DK                # 128 head dims per pair
HDC = NPAIRS * HD          # 256 head dims per core
KT_ = F // 128             # 8 contraction tiles for projections
TW = 512                   # t1 window (query cols per PSUM ctx tile)
NW = T // TW               # 4 windows
NT2 = T // 128             # 16 t2 tiles
NCH = T // 512             # 4 512-token chunks per matrix

f32 = mybir.dt.float32
f32r = mybir.dt.float32r
bf16 = mybir.dt.bfloat16
fp16 = mybir.dt.float16
EXP = mybir.ActivationFunctionType.Exp
MULT = mybir.AluOpType.mult

MODE = os.environ.get("MHA_MODE", "fp16")


def build_nc(include_bias: bool, mode: str = MODE):
    mdt = {"bf16": bf16, "fp16": fp16}.get(mode, f32r)
    nc = bacc.Bacc("TRN2", target_bir_lowering=False)

    # x^T pre-tiled on the host to [128, KT_, T] so one dma_start moves a
    # whole 512-token chunk (all feature tiles) with a simple 3D AP
    xqT = nc.dram_tensor("xqT", [128, KT_, T], mdt, kind="ExternalInput")
    xkT = nc.dram_tensor("xkT", [128, KT_, T], mdt, kind="ExternalInput")
    xvT = nc.dram_tensor("xvT", [128, KT_, T], mdt, kind="ExternalInput")
    wq = nc.dram_tensor("wq", [F, HDC], mdt, kind="ExternalInput")
    wk = nc.dram_tensor("wk", [F, HDC], mdt, kind="ExternalInput")
    wv = nc.dram_tensor("wv", [F, HDC], mdt, kind="ExternalInput")
    wo = nc.dram_tensor("wo", [HDC, F], mdt, kind="ExternalInput")
    ident_in = nc.dram_tensor("ident", [128, 128], mdt, kind="ExternalInput")
    # sel[:, c*64:(c+1)*64] = e_c selector (row c ones) for the recip bcast
    sel_in = nc.dram_tensor("sel", [8, 8 * 64], f32r, kind="ExternalInput")
    identr_in = nc.dram_tensor("identr", [128, 128], f32, kind="ExternalInput")
    if include_bias:
        bq = nc.dram_tensor("bq", [1, HDC], mdt, kind="ExternalInput")
        bk = nc.dram_tensor("bk", [1, HDC], mdt, kind="ExternalInput")
        bv = nc.dram_tensor("bv", [1, HDC], mdt, kind="ExternalInput")
    odt = mdt if mdt != f32r else f32
    out = nc.dram_tensor("out", [T, F], odt, kind="ExternalOutput")

    with tile.TileContext(nc) as tc:
        with (
            tc.tile_pool(name="const", bufs=1) as cpool,
            tc.tile_pool(name="xs", bufs=1) as xpool,
            tc.tile_pool(name="work", bufs=1) as wpool,
            tc.tile_pool(name="psum", bufs=1, space="PSUM") as psum,
        ):
            # ---- constants / weights resident in SBUF ----
            wq_s = cpool.tile([128, KT_, HDC], mdt, tag="wq")
            wk_s = cpool.tile([128, KT_, HDC], mdt, tag="wk")
            wv_s = cpool.tile([128, KT_, HDC], mdt, tag="wv")
            wo_s = cpool.tile([128, NPAIRS, F], mdt, tag="wo")
            ident = cpool.tile([128, 128], mdt, tag="ident")
            sel = cpool.tile([8, 8 * 64], f32r, tag="sel")
            identr = cpool.tile([128, 128], f32, tag="identr")
            xq_t = xpool.tile([128, KT_, T], mdt, tag="xq")
            xk_t = xpool.tile([128, KT_, T], mdt, tag="xk")
            xv_t = xpool.tile([128, KT_, T], mdt, tag="xv")
            xsrc = {"q": (xqT, xq_t), "k": (xkT, xk_t), "v": (xvT, xv_t)}

            def loadx(which, c, eng):
                """One DMA: 512-token chunk c of matrix `which`."""
                src, dst = xsrc[which]
                tsl = slice(c * 512, (c + 1) * 512)
                eng.dma_start(dst[:, :, tsl], src[:, :, tsl])

            # ---- input DMA schedule: issue order = arrival order ----
            # Issued before anything else so both HWDGE rings start
            # streaming immediately. Window0 gate: wk, wq, xk chunk0, xq
            # chunk0; K chunks and wv/xv0 next (their S/PV deadlines come
            # first), activation-tail chunks and wo last. The scalar ring
            # only carries these early loads (its queue drains before the
            # first exp); afterwards ScalarE carries only ACTIVATEs.
            nc.sync.dma_start(wk_s[:], wk.rearrange("(k p) m -> p k m", p=128))
            nc.scalar.dma_start(wq_s[:], wq.rearrange("(k p) m -> p k m", p=128))
            loadx("k", 0, nc.sync)
            loadx("q", 0, nc.scalar)
            nc.scalar.dma_start(ident[:], ident_in[:])
            nc.scalar.dma_start(wv_s[:], wv.rearrange("(k p) m -> p k m", p=128))
            loadx("v", 0, nc.sync)
            loadx("k", 1, nc.sync)
            loadx("k", 2, nc.scalar)
            loadx("v", 1, nc.sync)
            loadx("k", 3, nc.sync)
            loadx("v", 2, nc.scalar)
            loadx("q", 1, nc.scalar)
            nc.scalar.dma_start(sel[:], sel_in[:])
            nc.scalar.dma_start(identr[:], identr_in[:])
            loadx("v", 3, nc.sync)
            loadx("q", 2, nc.sync)
            loadx("q", 3, nc.sync)
            nc.sync.dma_start(wo_s[:], wo.rearrange("(c p) m -> p c m", p=128))

            with nc.allow_low_precision(reason="matmul operand rounding"):
                # prefetch the Exp table off the critical path
                warm_in = wpool.tile([1, 32], f32, tag="warm_i")
                warm_out = wpool.tile([1, 32], f32, tag="warm_o")
                nc.vector.memset(warm_in[:], 0.0)
                nc.scalar.activation(warm_out[:], warm_in[:], EXP)
                # HAM warm-up fodder: PE matmuls on memset data hold the
                # clock gate at 8/8 while the first input chunks stream in
                wma = wpool.tile([128, 2], mdt, tag="wma")
                wmb = wpool.tile([128, 512], mdt, tag="wmb")
                nc.vector.memset(wma[:], 0.0)
                nc.vector.memset(wmb[:], 0.0)
                # ones column pair for V1 (written into cols 64 and 129)
                onescol_f = wpool.tile([128, 2], f32, tag="c_f2")
                nc.vector.memset(onescol_f[:], 1.0)
                onescol = cpool.tile([128, 2], mdt, tag="onescol")
                nc.vector.tensor_copy(onescol[:], onescol_f[:])
                if include_bias:
                    bq_s = cpool.tile([1, HDC], mdt, tag="bq")
                    bk_s = cpool.tile([1, HDC], mdt, tag="bk")
                    bv_s = cpool.tile([1, HDC], mdt, tag="bv")
                    nc.sync.dma_start(bq_s[:], bq[:])
                    nc.sync.dma_start(bk_s[:], bk[:])
                    nc.sync.dma_start(bv_s[:], bv[:])
                    onesrow_f = wpool.tile([1, 512], f32, tag="c_f3")
                    nc.vector.memset(onesrow_f[:], 1.0)
                    onesrow = cpool.tile([1, 512], mdt, tag="onesrow")
                    nc.vector.tensor_copy(onesrow[:], onesrow_f[:])

            # HAM warm-up: ~4us of throwaway matmuls while DMA streams
            warm_ps = psum.tile([128, 512], f32, tag="ps", bufs=1)
            for _ in range(24):
                nc.tensor.matmul(warm_ps[0:2, :], wma[:], wmb[:],
                                 start=True, stop=True)

            # per-pair persistent tiles
            qt = {}; kt = {}; vt = {}; v1 = {}; ctxT = {}
            for p in range(NPAIRS):
                qt[p] = wpool.tile([HD, T], mdt, tag="qt", bufs=2,
                                   name=f"qt{p}")
                kt[p] = wpool.tile([HD, T], mdt, tag="kt", bufs=2,
                                   name=f"kt{p}")
                vt[p] = wpool.tile([HD, T], mdt, tag="vt", bufs=2,
                                   name=f"vt{p}")
                v1[p] = wpool.tile([128, NT2, 2 * 65], mdt, tag="v1", bufs=2,
                                   name=f"v1{p}")
                ctxT[p] = wpool.tile([HD, T], mdt, tag="ctxT", bufs=2,
                                     name=f"ctxT{p}")

            W_MATS = {"q": wq_s, "k": wk_s, "v": wv_s}
            DSTS = {"q": qt, "k": kt, "v": vt}
            XTS = {"q": xq_t, "k": xk_t, "v": xv_t}

            def proj_chunk(which, p, n):
                """Projection `which`, pair p, 512-token chunk n; yields
                after every matmul so drip granularity is ~215ns. Uses the
                long-lived PSUM bank; callers keep at most one chunk in
                flight."""
                w_s = W_MATS[which]
                dst = DSTS[which][p]
                xts = XTS[which]
                csl = slice(p * HD, (p + 1) * HD)
                ps = psum.tile([128, 512], f32, tag="pp", bufs=1)
                sl = slice(n * 512, (n + 1) * 512)
                for k in range(KT_):
                    nc.tensor.matmul(
                        ps[:], w_s[:, k, csl], xts[:, k, sl],
                        start=(k == 0),
                        stop=(k == KT_ - 1) and not include_bias,
                    )
                    yield
                if include_bias:
                    nc.tensor.matmul(ps[:], {"q": bq_s, "k": bk_s,
                                             "v": bv_s}[which][:, csl],
                                     onesrow[:], start=False, stop=True)
                with nc.allow_low_precision(reason="rounding"):
                    nc.vector.tensor_copy(dst[:, sl], ps[:])
                yield

            def gen_proj(which, p, chunks=tuple(range(NCH))):
                for n in chunks:
                    for _ in proj_chunk(which, p, n):
                        yield

            def v1_tile(p, tcid):
                """Token-major V (+ones cols) via a PE transpose of V^T."""
                pt = psum.tile([128, 128], mdt, tag="ps", bufs=1)
                tsl = slice(tcid * 128, (tcid + 1) * 128)
                nc.tensor.transpose(pt[:], vt[p][:, tsl], ident[:])
                with nc.allow_low_precision(reason="rounding"):
                    nc.vector.tensor_copy(v1[p][:, tcid, 0:64], pt[:, 0:64])
                    nc.vector.tensor_copy(v1[p][:, tcid, 65:129],
                                          pt[:, 64:128])
                    nc.vector.tensor_copy(v1[p][:, tcid, 64:130:65],
                                          onescol[:])

            def gen_v1(p):
                for tcid in range(NT2):
                    v1_tile(p, tcid)
                    yield

            def gen_vw(p):
                """V projection chunk-interleaved with its v1 transposes,
                so PV(t2) dependencies complete in t2 order."""
                for c in range(NCH):
                    for _ in proj_chunk("v", p, c):
                        yield
                    for tcid in range(4 * c, 4 * c + 4):
                        v1_tile(p, tcid)
                        yield

            def gen_oproj(lo, hi, tag="ps", bufs=1):
                """Output projection token-chunks [lo, hi), both pairs
                accumulated in PSUM so each token chunk is one partial.
                The final window passes tag="st": the score-tile banks are
                dead by then and give double-buffering."""
                for tcid in range(lo, hi):
                    tsl = slice(tcid * 128, (tcid + 1) * 128)
                    ob = wpool.tile([128, F], odt, tag="ob", bufs=2)
                    for half in range(2):
                        po = psum.tile([128, 512], f32, tag=tag, bufs=bufs)
                        fsl = slice(half * 512, (half + 1) * 512)
                        nc.tensor.matmul(po[:], ctxT[0][:, tsl],
                                         wo_s[:, 0, fsl],
                                         start=True, stop=False)
                        yield
                        nc.tensor.matmul(po[:], ctxT[1][:, tsl],
                                         wo_s[:, 1, fsl],
                                         start=False, stop=True)
                        with nc.allow_low_precision(reason="partial sums"):
                            nc.vector.tensor_copy(ob[:, fsl], po[:])
                        nc.sync.dma_start(out[tsl, fsl], ob[:, fsl])
                        yield

            pending = deque()

            def consume(k):
                done = 0
                while pending and done < k:
                    try:
                        next(pending[0])
                        done += 1
                    except StopIteration:
                        pending.popleft()

            def drain(gen):
                """Force a generator to completion (deadline safety net);
                also removes it from pending if queued."""
                for _ in gen:
                    pass
                try:
                    pending.remove(gen)
                except ValueError:
                    pass

            def gen_norm(p, n, cd2, rca, rcb):
                """Drip-able remainder of the window normalization: the
                sums rows are transposed into columns with tiny PE
                transposes so the reciprocal runs across lanes, transposed
                back, and broadcast to the head-half partitions via
                selector matmuls; ctxT = ctx * recip(sums)."""
                nch = TW // 128  # 128-col chunks in the window
                wsl = slice(n * TW, (n + 1) * TW)
                pts = psum.tile([128, 2 * nch], f32, tag="ps", bufs=1)
                for h, rch in ((0, rca), (1, rcb)):
                    for c in range(nch):
                        nc.tensor.transpose(pts[:, h * nch + c:h * nch + c + 1],
                                            rch[0:1, c * 128:(c + 1) * 128],
                                            identr[0:1, 0:1])
                rcc = wpool.tile([128, 2 * nch], f32, tag="rcc", bufs=2)
                nc.vector.reciprocal(rcc[:], pts[:])
                yield
                pr = psum.tile([2 * nch, 128], f32, tag="ps", bufs=1)
                nc.tensor.transpose(pr[:], rcc[:], identr[:])
                rcr = wpool.tile([2 * nch, 128], f32r, tag="rcr", bufs=2)
                with nc.allow_low_precision(reason="rounding"):
                    nc.vector.tensor_copy(rcr[:], pr[:])
                yield
                scp = psum.tile([128, TW], f32, tag="ps", bufs=1)
                for c in range(nch):
                    nc.tensor.matmul(scp[:, c * 128:(c + 1) * 128],
                                     sel[0:2 * nch, c * 128:(c + 1) * 128],
                                     rcr[:], start=True, stop=True)
                sc = wpool.tile([128, TW], f32, tag="sc", bufs=2)
                nc.vector.tensor_copy(sc[:], scp[:])
                yield
                with nc.allow_low_precision(reason="rounding"):
                    nc.vector.tensor_tensor(ctxT[p][:, wsl], cd2[:],
                                            sc[:], MULT)
                yield

            def norm_start(p, n, ctx0, ctx1):
                """Immediate part of normalization: copy ctx + sums off
                PSUM (frees the banks so the next window's PV never
                stalls); the arithmetic drips via gen_norm."""
                rca = wpool.tile([1, TW], f32, tag="rca", bufs=2)
                nc.vector.tensor_copy(rca[:], ctx0[64:65, :])
                rcb = wpool.tile([1, TW], f32, tag="rcb", bufs=2)
                nc.vector.tensor_copy(rcb[:], ctx1[64:65, :])
                cd2 = wpool.tile([128, TW], f32, tag="cd", bufs=2)
                nc.vector.tensor_copy(cd2[0:64, :], ctx0[0:64, :])
                nc.vector.tensor_copy(cd2[64:128, :], ctx1[0:64, :])
                return gen_norm(p, n, cd2, rca, rcb)

            def s_pair(p, n, t2):
                """Scores for both heads of pair p, t2 tile, window n.
                Row-tiled at (0,0)/(64,0) so the two 64-deep matmuls run
                concurrently; outputs land in separate PSUM banks."""
                s = psum.tile([128, 2 * TW], f32, tag="st", bufs=2)
                t2sl = slice(t2 * 128, (t2 + 1) * 128)
                qsl = slice(n * TW, (n + 1) * TW)
                nc.tensor.matmul(s[:, 0:TW], kt[p][0:64, t2sl],
                                 qt[p][0:64, qsl], start=True, stop=True,
                                 tile_position=(0, 0))
                nc.tensor.matmul(s[:, TW:2 * TW], kt[p][64:128, t2sl],
                                 qt[p][64:128, qsl], start=True, stop=True,
                                 tile_position=(64, 0))
                return s

            def attn_body(p, n, s, nxt, predrip=None, fill=2, force=None):
                """One window of attention. `s` is the pre-emitted first
                score tile. Software-pipelined: S(t2+1) is emitted before
                PV(t2) so exp overlaps PE work; the NEXT window's first S
                is emitted two iterations early so its exp starts with
                zero gap. `predrip(t2)` emits must-land-now work before
                S(t2+1); `fill` caps pending yields per iteration;
                `force` (at t2==13) drains generators the next window's
                hoisted S will read from."""
                ctx0 = psum.tile([65, TW], f32, tag="ctx0", bufs=1)
                ctx1 = psum.tile([65, TW], f32, tag="ctx1", bufs=1)
                s_next = None
                for t2 in range(NT2):
                    es = wpool.tile([128, 2 * TW], mdt, tag="es", bufs=12)
                    with nc.allow_low_precision(reason="rounding"):
                        nc.scalar.activation(es[:], s[:], EXP, scale=0.125)
                    if predrip is not None:
                        predrip(t2)
                    if t2 == 13 and force is not None:
                        for g in force:
                            drain(g)
                    if t2 < NT2 - 1:
                        s = s_pair(p, n, t2 + 1)
                    if t2 == NT2 - 2 and nxt is not None:
                        s_next = s_pair(nxt[0], nxt[1], 0)
                    if t2 < NT2 - 2:
                        consume(min(fill, 1) if fill else 0)
                    nc.tensor.matmul(ctx0[:], v1[p][:, t2, 0:65],
                                     es[:, 0:TW],
                                     start=(t2 == 0), stop=(t2 == NT2 - 1))
                    nc.tensor.matmul(ctx1[:], v1[p][:, t2, 65:130],
                                     es[:, TW:2 * TW],
                                     start=(t2 == 0), stop=(t2 == NT2 - 1))
                    if t2 < NT2 - 1:
                        consume(max(fill - 1, 0))
                return ctx0, ctx1, s_next

            # ---- emission schedule ----
            # window 0 gate: K chunk0 + Q chunk0 of pair 0 (burst).
            for _ in gen_proj("k", 0, chunks=(0,)):
                pass
            for _ in gen_proj("q", 0, chunks=(0,)):
                pass

            gk0 = {c: gen_proj("k", 0, chunks=(c,)) for c in (1, 2, 3)}
            gq0 = {c: gen_proj("q", 0, chunks=(c,)) for c in (1, 2, 3)}
            gv0 = gen_vw(0)
            gk1 = gen_proj("k", 1)
            gq1 = {c: gen_proj("q", 1, chunks=(c,)) for c in range(NCH)}
            gv1 = gen_vw(1)

            def predrip0(t2):
                # paced to DMA arrival: V chunk c's projection and first
                # v1 transpose must all precede PV(4c) (emission order is
                # dataflow order), the remaining transposes ride one per
                # iter; K chunk c lands after its DMA and before its S
                # tiles; the pair-0 Q1 tail once xq chunk 1 has arrived.
                for _ in range(10 if t2 % 4 == 0 else 1):
                    next(gv0, None)
                if t2 in (1, 5, 9):
                    kc = gk0[{1: 1, 5: 2, 9: 3}[t2]]
                    for _ in range(5):
                        next(kc, None)
                elif t2 in (2, 3, 6, 7, 10, 11):
                    kc = gk0[{2: 1, 3: 1, 6: 2, 7: 2, 10: 3, 11: 3}[t2]]
                    drain(kc) if t2 % 4 == 3 else [next(kc, None)
                                                   for _ in range(2)]
                if t2 in (9, 10, 11):
                    for _ in range(3):
                        next(gq0[1], None)
                if t2 == 12:
                    drain(gq0[1])

            def predrip1(t2):
                # pair-1 V chunk + v1 transposes: full chunk and first
                # transpose before PV(4c), the rest one per iter
                for _ in range(10 if t2 % 4 == 0 else 1):
                    next(gv1, None)

            # pending drips in deadline order through the pair-0 windows:
            # qt chunk n is needed by window n's hoisted first S (end of
            # window n-1); kt[1]/qt[1] chunk0 by the pair-1 hoist at the
            # end of window (0,3); the rest inside pair 1.
            pending.append(gq0[2])
            pending.append(gk1)
            pending.append(gq0[3])
            pending.append(gq1[0])
            pending.append(gq1[1])
            pending.append(gq1[2])
            pending.append(gq1[3])

            tc_per_w = T // 128 // NW
            sched = [(p, n) for p in range(NPAIRS) for n in range(NW)]
            FORCES = {(0, 1): lambda: [gq0[2]], (0, 2): lambda: [gq0[3]],
                      (0, 3): lambda: [gk1, gq1[0]],
                      (1, 0): lambda: [gq1[1]], (1, 1): lambda: [gq1[2]],
                      (1, 2): lambda: [gq1[3]]}
            s = s_pair(0, 0, 0)
            for i, (p, n) in enumerate(sched):
                nxt = sched[i + 1] if i + 1 < len(sched) else None
                ctx0, ctx1, s = attn_body(
                    p, n, s, nxt,
                    predrip=predrip0 if i == 0 else
                            (predrip1 if (p, n) == (1, 0) else None),
                    fill=0 if i == 0 else (8 if (p, n) == (1, NW - 1)
                                           else 2),
                    force=FORCES.get((p, n), lambda: [])())
                pending.appendleft(norm_start(p, n, ctx0, ctx1))
                if p == NPAIRS - 1:
                    if n == NW - 1:
                        pending.append(gen_oproj(n * tc_per_w,
                                                 (n + 1) * tc_per_w,
                                                 tag="st", bufs=2))
                    else:
                        pending.append(gen_oproj(n * tc_per_w,
                                                 (n + 1) * tc_per_w))
            while pending:
                try:
                    next(pending[0])
                except StopIteration:
                    pending.popleft()

    nc.compile()
    return nc


_CACHE = {}


def _get_nc(include_bias: bool):
    key = (include_bias, MODE)
    if key not in _CACHE:
        _CACHE[key] = build_nc(include_bias)
    return _CACHE[key]


def _reference_fallback(query, key_, value, mask, Wq, bq, Wk, bk, Wv, bv, Wo, bo):
    """Plain numpy fallback (only used if the mask is not all-ones)."""
    q = (query @ Wq + bq).reshape(B, T, H, DK).transpose(0, 2, 1, 3)
    k = (key_ @ Wk + bk).reshape(B, T, H, DK).transpose(0, 2, 1, 3)
    v = (value @ Wv + bv).reshape(B, T, H, DK).transpose(0, 2, 1, 3)
    scores = np.einsum("bhqd,bhkd->bhqk", q, k) / np.sqrt(np.float32(DK))
    scores = np.where(mask[:, None, :, :] > 0, scores,
                      np.float32(-10000.0)).astype(np.float32)
    scores -= scores.max(axis=-1, keepdims=True)
    e = np.exp(scores)
    attn = e / e.sum(axis=-1, keepdims=True)
    x = np.einsum("bhqk,bhkd->bhqd", attn, v)
    x = x.transpose(0, 2, 1, 3).reshape(B, T, F)
    return (x @ Wo + bo).astype(np.float32)


def _mdt_np(arr):
    if MODE == "bf16":
        import ml_dtypes
        return np.ascontiguousarray(arr).astype(ml_dtypes.bfloat16)
    if MODE == "fp16":
        return np.ascontiguousarray(arr).astype(np.float16)
    return np.ascontiguousarray(arr)


def _xtile(xb):
    """[T, F] activation -> [128, KT_, T] tiled transpose."""
    return _mdt_np(xb.T.reshape(KT_, 128, T).transpose(1, 0, 2))


def make_in_maps(query, key_, value, Wq, Wk, Wv, Wo, bq=None, bk=None, bv=None):
    xqT = [_xtile(query[b]) for b in range(B)]
    xkT = [_xtile(key_[b]) for b in range(B)]
    xvT = [_xtile(value[b]) for b in range(B)]
    ident = _mdt_np(np.eye(128, dtype=np.float32))
    identr = np.eye(128, dtype=np.float32)
    # sel[(j//64)*nch + c, c*128 + j] = 1: selector that broadcasts the
    # per-chunk reciprocal rows to the right head-half partitions
    nch = TW // 128
    sel = np.zeros((8, 8 * 64), np.float32)
    for c in range(nch):
        sel[c, c * 128:c * 128 + 64] = 1.0
        sel[nch + c, c * 128 + 64:(c + 1) * 128] = 1.0
    in_maps = []
    for c in range(NCORES):
        b, g = c // NGROUPS, c % NGROUPS
        csl = slice(g * HDC, (g + 1) * HDC)
        m = {
            "xqT": xqT[b], "xkT": xkT[b], "xvT": xvT[b], "ident": ident,
            "sel": sel, "identr": identr,
            "wq": _mdt_np(Wq[:, csl]),
            "wk": _mdt_np(Wk[:, csl]),
            "wv": _mdt_np(Wv[:, csl]),
            "wo": _mdt_np(Wo[csl, :]),
        }
        if bq is not None:
            m["bq"] = _mdt_np(bq[None, csl])
            m["bk"] = _mdt_np(bk[None, csl])
            m["bv"] = _mdt_np(bv[None, csl])
        in_maps.append(m)
    return in_maps


def kernel(**inputs) -> np.ndarray:
    query = np.asarray(inputs["query"], np.float32)
    key_ = np.asarray(inputs.get("key_", inputs.get("key")), np.float32)
    value = np.asarray(inputs["value"], np.float32)
    mask = np.asarray(inputs["mask"])
    Wq, bq = np.asarray(inputs["Wq"], np.float32), np.asarray(inputs["bq"], np.float32)
    Wk, bk = np.asarray(inputs["Wk"], np.float32), np.asarray(inputs["bk"], np.float32)
    Wv, bv = np.asarray(inputs["Wv"], np.float32), np.asarray(inputs["bv"], np.float32)
    Wo, bo = np.asarray(inputs["Wo"], np.float32), np.asarray(inputs["bo"], np.float32)

    if not (mask > 0).all():
        return _reference_fallback(query, key_, value, mask,
                                   Wq, bq, Wk, bk, Wv, bv, Wo, bo)

    include_bias = bool(np.any(bq) or np.any(bk) or np.any(bv))
    nc = _get_nc(include_bias)
    if include_bias:
        in_maps = make_in_maps(query, key_, value, Wq, Wk, Wv, Wo, bq, bk, bv)
    else:
        in_maps = make_in_maps(query, key_, value, Wq, Wk, Wv, Wo)

    res = run_bass_kernel_spmd(nc, in_maps, core_ids=list(range(NCORES)))
    total = np.zeros((B, T, F), np.float32)
    for c in range(NCORES):
        total[c // NGROUPS] += res.results[c]["out"]
    return (total + bo).astype(np.float32)
